# revision 27
# baseline (speedup 1.0000x reference)
"""Multi-head attention forward on 8 Trainium2 NeuronCores.

Problem: B=4, S=2048, E=1024, H=16, D=64 (fp32 in/out).

Sharding: 8 cores = (batch b, sequence half). Each core handles the full
key/value sequence of its batch (K/V projections computed redundantly by the
2 cores sharing a batch) and 1024 query rows, so outputs are disjoint and no
collective is needed. Inputs are host-rolled so each core's query rows are
rows 0:1024 of its x — softmax over keys is permutation invariant, so rolling
the key axis does not change the result. x^T is pre-transposed on the host.

One software-pipelined stream keeps the Scalar engine's exp (~280us of
ACTIVATE, the second roofline after the PE) running under all PE work:

  per head pair j, 16 iterations of (j, qc), merge_i =
    [ per k-block: both heads' scores MMs (row groups 0:64/64:128,
      back-to-back into one PSUM tile -> one exp ACT over both heads)
      interleaved with ctx MMs of iteration i-1 ]
  K proj for pair j+1, the V projection (kb-granular, just in time for the
  first ctx), Q's second half and the first-half output projection are all
  woven into the merge stream as `extra` slots.

PSUM: scores 3x[128,1024] (depth-3 so MMs never couple to ACT latency),
ctx pair combined in 1x[128,1024] (head per bank), projections packed in
pairs through the scores pool.
"""

import os
import sys
import types

import numpy as np

sys.path.insert(0, "/opt/trn_rl_repo")

B, S, E, H = 4, 2048, 1024, 16
D = E // H          # 64
Q = S // 2          # query rows per core
NCORES = 8

_compiled = None


def _install_prof_hook():
    try:
        import antenv.axon_hooks  # noqa: F401
        return
    except ImportError:
        pass
    try:
        import antenv
        from trn_agent_boot.trn_boot import _ntff_profile_via_ctypes
    except ImportError:
        return
    mod = types.ModuleType("antenv.axon_hooks")
    mod._hook = None
    mod.set_axon_ntff_profile_hook = lambda h: setattr(mod, "_hook", h)
    mod.get_axon_ntff_profile_hook = lambda: mod._hook
    sys.modules["antenv.axon_hooks"] = mod
    antenv.axon_hooks = mod
    try:
        mod._hook = _ntff_profile_via_ctypes("/opt/axon/libaxon_pjrt.so")
    except Exception:
        mod._hook = None


def _build():
    from contextlib import ExitStack

    from concourse import bacc
    import concourse.mybir as mybir
    from concourse import tile_utils
    from concourse.tile import TileContext

    tile_utils.max_sbuf_usage = 207 * 1024

    F32 = mybir.dt.float32
    BF16 = mybir.dt.bfloat16
    Exp = mybir.ActivationFunctionType.Exp

    nc = bacc.Bacc("TRN2", target_bir_lowering=False, debug=False)

    xt = nc.dram_tensor("xt", [E, S], BF16, kind="ExternalInput")   # x^T
    wq = nc.dram_tensor("wq", [E, E], BF16, kind="ExternalInput")
    wk = nc.dram_tensor("wk", [E, E], BF16, kind="ExternalInput")
    wv = nc.dram_tensor("wv", [E, E], BF16, kind="ExternalInput")
    wo = nc.dram_tensor("wo", [E, E], BF16, kind="ExternalInput")
    y = nc.dram_tensor("y", [Q, E], F32, kind="ExternalOutput")

    xt_v = xt.ap().rearrange("(eb p) s -> p eb s", p=128)           # [128, 8, 2048]
    wq_v = wq.ap().rearrange("(eb p) n -> p eb n", p=128)
    wk_v = wk.ap().rearrange("(eb p) n -> p eb n", p=128)
    wv_v = wv.ap().rearrange("(eb p) n -> p eb n", p=128)
    wo_v = wo.ap().rearrange("(eb p) n -> p eb n", p=128)
    y_v = y.ap().rearrange("(sb p) e -> sb p e", p=128)             # [8, 128, 1024]

    EB = E // 128        # 8 e-chunks
    SB = S // 128        # 16 s blocks (keys)
    KB = S // 128        # 16 key blocks

    with TileContext(nc) as tc:
        with ExitStack() as es:
            xtp = es.enter_context(tc.tile_pool(name="xt", bufs=1))
            kTp = es.enter_context(tc.tile_pool(name="kT", bufs=1))
            qTp = es.enter_context(tc.tile_pool(name="qT", bufs=1))
            vp = es.enter_context(tc.tile_pool(name="vA", bufs=1))
            ctxp = es.enter_context(tc.tile_pool(name="ctx", bufs=1))
            attnp = es.enter_context(tc.tile_pool(name="attn", bufs=3))
            wkqp = es.enter_context(tc.tile_pool(name="wkq", bufs=2))
            wqp2 = es.enter_context(tc.tile_pool(name="wq2", bufs=2))
            wvp = es.enter_context(tc.tile_pool(name="wvp", bufs=2))
            ytp = es.enter_context(tc.tile_pool(name="yt", bufs=1))
            nrmp = es.enter_context(tc.tile_pool(name="nrm", bufs=1))
            stgp = es.enter_context(tc.tile_pool(name="stg", bufs=2))
            psA = es.enter_context(tc.tile_pool(name="psA", bufs=2, space="PSUM"))
            psB = es.enter_context(tc.tile_pool(name="psB", bufs=2, space="PSUM"))

            xts = xtp.tile([128, EB, S], BF16)       # x^T  [e, s]
            kT = kTp.tile([128, EB, S], BF16)        # K^T  [n, s]
            qT = qTp.tile([128, EB, Q], BF16)        # Q^T  [n, q]
            vA = vp.tile([128, SB, H, D + 1], BF16)  # V | ones column
            ctx = ctxp.tile([128, EB, Q], BF16)      # ctx^T [e, q]

            # x^T arrives via 4 plain chunk DMAs (contiguous rows)
            for scc in range(4):
                nc.sync.dma_start(xts[:, :, scc * 512:(scc + 1) * 512],
                                  xt_v[:, :, scc * 512:(scc + 1) * 512])
            nc.gpsimd.memset(vA[:, :, :, D], 1.0)    # ones column (all heads)

            inv_sqrt_d = 1.0 / float(np.sqrt(D))

            def emit_qproj_pair(qc, nbp):
                """Q^T for column pair (2*nbp, 2*nbp+1), query chunk qc."""
                nb = 2 * nbp
                wt = wqp2.tile([128, EB, 256], BF16, tag="wq2",
                               name=f"wtq{qc}_{nbp}")
                nc.gpsimd.dma_start(wt[:], wq_v[:, :, nb * 128:(nb + 2) * 128])
                ps = psA.tile([128, 1024], F32, tag="sc", name=f"pq{qc}_{nbp}")
                for half in range(2):
                    for eb in range(EB):
                        nc.tensor.matmul(
                            ps[:, half * 512:(half + 1) * 512],
                            wt[:, eb, half * 128:(half + 1) * 128],
                            xts[:, eb, qc * 512:(qc + 1) * 512],
                            start=(eb == 0), stop=(eb == EB - 1))
                nc.vector.tensor_copy(
                    qT[:, nb:nb + 2, qc * 512:(qc + 1) * 512],
                    ps.rearrange("p (a b) -> p a b", a=2))

            ktiles = {}

            def kproj_half(nb, sh):
                """K^T column block nb, key half sh (one [128,1024] tile)."""
                def fn():
                    if sh == 0:
                        wt = wkqp.tile([128, EB, 128], BF16, tag="wkq",
                                       name=f"wtk{nb}")
                        nc.gpsimd.dma_start(
                            wt[:], wk_v[:, :, nb * 128:(nb + 1) * 128])
                        ktiles[nb] = wt
                    wt = ktiles[nb]
                    ps = psA.tile([128, 1024], F32, tag="sc", name=f"pk{nb}_{sh}")
                    for half in range(2):
                        sc = 2 * sh + half
                        for eb in range(EB):
                            nc.tensor.matmul(
                                ps[:, half * 512:(half + 1) * 512],
                                wt[:, eb, :],
                                xts[:, eb, sc * 512:(sc + 1) * 512],
                                start=(eb == 0), stop=(eb == EB - 1))
                    nc.vector.tensor_copy(
                        kT[:, nb, sh * 1024:(sh + 1) * 1024], ps[:])
                return fn

            def emit_kproj(nb):
                kproj_half(nb, 0)()
                kproj_half(nb, 1)()

            def load_wv(nc2):
                def fn():
                    wvt = []
                    for ebh in range(2):
                        wvh = wvp.tile([128, 4, 512], BF16, tag="wv",
                                       name=f"wv{nc2}_{ebh}")
                        nc.gpsimd.dma_start(
                            wvh[:], wv_v[:, ebh * 4:(ebh + 1) * 4,
                                         nc2 * 512:(nc2 + 1) * 512])
                        wvt.append(wvh)
                    wo_tiles[f"v{nc2}"] = wvt
                return fn

            def vproj_pair(nc2, sbp):
                """V for s-block pair (2*sbp, 2*sbp+1), head half nc2."""
                def fn():
                    wvt = wo_tiles[f"v{nc2}"]
                    ps = psA.tile([128, 1024], F32, tag="sc",
                                  name=f"pv{nc2}_{sbp}")
                    for si in range(2):
                        sb = 2 * sbp + si
                        for eb in range(EB):
                            nc.tensor.matmul(
                                ps[:, si * 512:(si + 1) * 512],
                                xts[:, eb, sb * 128:(sb + 1) * 128],
                                wvt[eb // 4][:, eb % 4, :],
                                start=(eb == 0), stop=(eb == EB - 1))
                    nc.vector.tensor_copy(
                        vA[:, 2 * sbp:2 * sbp + 2, nc2 * 8:(nc2 + 1) * 8, 0:D],
                        ps.rearrange("p (s h d) -> p s h d", s=2, d=D))
                return fn

            wo_tiles = {}

            def load_wo(nc2):
                def fn():
                    wot = []
                    for ebh in range(2):
                        woh = wvp.tile([128, 4, 512], BF16, tag="wv",
                                       name=f"wo{nc2}_{ebh}")
                        nc.gpsimd.dma_start(
                            woh[:], wo_v[:, ebh * 4:(ebh + 1) * 4,
                                         nc2 * 512:(nc2 + 1) * 512])
                        wot.append(woh)
                    wo_tiles[nc2] = wot
                return fn

            def oproj_pair(nc2, sbp):
                """y rows for s-block pair (2*sbp, 2*sbp+1), column half nc2."""
                def fn():
                    wot = wo_tiles[nc2]
                    ps = psA.tile([128, 1024], F32, tag="sc",
                                  name=f"yp{nc2}_{sbp}")
                    for si in range(2):
                        sb = 2 * sbp + si
                        for eb in range(EB):
                            nc.tensor.matmul(
                                ps[:, si * 512:(si + 1) * 512],
                                ctx[:, eb, sb * 128:(sb + 1) * 128],
                                wot[eb // 4][:, eb % 4, :],
                                start=(eb == 0), stop=(eb == EB - 1))
                    yt = ytp.tile([128, 2, 512], F32)
                    nc.vector.tensor_copy(
                        yt[:], ps.rearrange("p (a b) -> p a b", a=2))
                    for si in range(2):
                        nc.sync.dma_start(
                            y_v[2 * sbp + si][:, nc2 * 512:(nc2 + 1) * 512],
                            yt[:, si, :])
                return fn

            def emit_merge(cur, prev_state, extra=(), slot_every=4,
                           slow_ctx=False):
                """Scores+exp for iteration `cur` (one [128,1024] tile and one
                ACT per k-block, both heads), ctx MMs of `prev_state`
                interleaved kb-by-kb; `extra` emitters run every `slot_every`
                k-blocks."""
                j, qc = cur
                qs = slice(qc * 512, (qc + 1) * 512)
                at = [attnp.tile([128, 8, 2, 512], BF16, tag="attn",
                                 name=f"at{j}_{qc}_{half}") for half in range(2)]
                if prev_state is not None:
                    (pj, pqc), pat, pcps = prev_state
                extra = list(extra)
                for kb in range(KB):
                    sps = psA.tile([128, 1024], F32, tag="sc",
                                   name=f"sc{j}_{qc}_{kb}")
                    for hh in range(2):           # back-to-back head pair
                        p0 = hh * 64
                        nc.tensor.matmul(
                            sps[:, hh * 512:(hh + 1) * 512],
                            kT[p0:p0 + 64, j, kb * 128:(kb + 1) * 128],
                            qT[p0:p0 + 64, j, qs],
                            start=True, stop=True)
                    nc.scalar.activation(
                        at[kb // 8][:, kb % 8, :, :]
                        .rearrange("p a b -> p (a b)"),
                        sps[:], Exp, scale=inv_sqrt_d)
                    # previous iteration's ctx at double rate in the first
                    # half so its attn tiles free mid-merge (the next
                    # iteration's exp reuses their buffers); merge 1 runs it
                    # kb-paced because V is still streaming in.
                    if prev_state is not None:
                        if slow_ctx:
                            cks = (kb,)
                        elif kb < 8:
                            cks = (2 * kb, 2 * kb + 1)
                        else:
                            cks = ()
                        for ck in cks:
                            for hh in range(2):
                                nc.tensor.matmul(
                                    pcps[0:D + 1, hh * 512:(hh + 1) * 512],
                                    vA[:, ck, 2 * pj + hh, :],
                                    pat[ck // 8][:, ck % 8, hh, :],
                                    start=(ck == 0), stop=(ck == KB - 1))
                        # normalize right after the last ctx MM so the DVE
                        # chain drains while this merge finishes (frees the
                        # ctx PSUM tile before the next merge needs it)
                        if cks and cks[-1] == KB - 1:
                            finish_ctx(prev_state)
                            prev_state = None
                    if extra and kb % slot_every == slot_every - 1:
                        extra.pop(0)()
                if prev_state is not None:
                    finish_ctx(prev_state)
                for fn in extra:
                    fn()
                cps = psB.tile([128, 1024], F32, tag="cx", name=f"cps{j}_{qc}")
                return (cur, at, cps)

            def finish_ctx(state):
                (pj, pqc), pat, pcps = state
                pqs = slice(pqc * 512, (pqc + 1) * 512)
                for hh in range(2):
                    cs = slice(hh * 512, (hh + 1) * 512)
                    den = nrmp.tile([1, 512], F32, tag="den")
                    nc.vector.tensor_copy(den[:], pcps[D:D + 1, cs])
                    nc.vector.reciprocal_approx_fast(den[:], den[:])
                    bcast = nrmp.tile([64, 512], F32, tag="bc")
                    nc.gpsimd.partition_broadcast(bcast[:], den[:])
                    if hh == 0:
                        nc.vector.tensor_mul(
                            ctx[0:64, pj, pqs], pcps[0:D, cs], bcast[:])
                    else:
                        stg = stgp.tile([64, 512], BF16, tag="stg")
                        nc.vector.tensor_mul(stg[:], pcps[0:D, cs], bcast[:])
                        nc.sync.dma_start(ctx[64:128, pj, pqs], stg[:])

            def run_ctx_only(state):
                (pj, pqc), pat, pcps = state
                for kb in range(KB):
                    for hh in range(2):
                        nc.tensor.matmul(
                            pcps[0:D + 1, hh * 512:(hh + 1) * 512],
                            vA[:, kb, 2 * pj + hh, :],
                            pat[kb // 8][:, kb % 8, hh, :],
                            start=(kb == 0), stop=(kb == KB - 1))
                finish_ctx(state)

            # -------------------- the pipeline --------------------
            _sc = nc.named_scope("pipe"); _sc.__enter__()
            load_wv(0)()
            emit_qproj_pair(0, 0)        # only nb 0,1 needed for scores(0,*)
            emit_kproj(0)

            iters = [(j, 0) for j in range(H // 2)] + \
                    [(j, 1) for j in range(H // 2)]

            # extras woven into merges: K proj just-in-time for the next
            # head pair, V in s-block pairs just ahead of the ctx consumers,
            # Q's second half, then the first-half output projection.
            extras = {
                0: [lambda: emit_qproj_pair(0, 1), kproj_half(1, 0),
                    vproj_pair(0, 0), lambda: emit_qproj_pair(0, 2),
                    kproj_half(1, 1), vproj_pair(0, 1),
                    lambda: emit_qproj_pair(0, 3), vproj_pair(0, 2)],
                1: [kproj_half(2, 0), vproj_pair(0, 3), vproj_pair(0, 4),
                    vproj_pair(0, 5), vproj_pair(0, 6), vproj_pair(0, 7),
                    kproj_half(2, 1), load_wv(1)],
                2: [kproj_half(3, 0), vproj_pair(1, 0), kproj_half(3, 1),
                    vproj_pair(1, 1), vproj_pair(1, 2)],
                3: [kproj_half(4, 0), vproj_pair(1, 3), kproj_half(4, 1),
                    vproj_pair(1, 4), vproj_pair(1, 5)],
                4: [vproj_pair(1, 6), vproj_pair(1, 7), kproj_half(5, 0),
                    kproj_half(5, 1)],
                5: [kproj_half(6, 0), kproj_half(6, 1)],
                6: [kproj_half(7, 0), lambda: emit_qproj_pair(1, 0),
                    kproj_half(7, 1), lambda: emit_qproj_pair(1, 1)],
                7: [lambda: emit_qproj_pair(1, 2), lambda: emit_qproj_pair(1, 3)],
                9: [load_wo(0), oproj_pair(0, 0)],
                10: [oproj_pair(0, 1)],
                11: [load_wo(1), oproj_pair(1, 0)],
                12: [oproj_pair(1, 1)],
            }
            state = emit_merge(iters[0], None, extras[0], slot_every=2)
            for i in range(1, len(iters)):
                state = emit_merge(iters[i], state, extras.get(i, ()),
                                   slot_every=2, slow_ctx=(i == 1))
            run_ctx_only(state)                         # ctx(7,1)

            # tail: y rows 512:1024 (wo nc2=1 still resident, then reload 0)
            oproj_pair(1, 2)()
            oproj_pair(1, 3)()
            load_wo(0)()
            oproj_pair(0, 2)()
            oproj_pair(0, 3)()
            _sc.__exit__(None, None, None)

    nc.compile()
    return nc


def kernel(x, Wq, Wk, Wv, Wo):
    global _compiled
    _install_prof_hook()
    import ml_dtypes
    from concourse import bass_utils

    if _compiled is None:
        _compiled = _build()
    nc = _compiled

    bf16 = ml_dtypes.bfloat16
    x = np.ascontiguousarray(x, dtype=np.float32)
    wq_b = np.ascontiguousarray(np.asarray(Wq, dtype=np.float32).astype(bf16))
    wk_b = np.ascontiguousarray(np.asarray(Wk, dtype=np.float32).astype(bf16))
    wv_b = np.ascontiguousarray(np.asarray(Wv, dtype=np.float32).astype(bf16))
    wo_b = np.ascontiguousarray(np.asarray(Wo, dtype=np.float32).astype(bf16))

    in_maps = []
    for c in range(NCORES):
        b, half = c // 2, c % 2
        xc = np.roll(x[b], -Q * half, axis=0) if half else x[b]
        in_maps.append({
            "xt": np.ascontiguousarray(xc.T.astype(bf16)),
            "wq": wq_b, "wk": wk_b, "wv": wv_b, "wo": wo_b,
        })

    trace = bool(int(os.environ.get("KERNEL_TRACE", "0")))
    res = bass_utils.run_bass_kernel_spmd(
        nc, in_maps, core_ids=list(range(NCORES)), trace=trace)
    kernel.last_result = res

    out = np.empty((B, S, E), dtype=np.float32)
    for c in range(NCORES):
        b, half = c // 2, c % 2
        out[b, half * Q:(half + 1) * Q] = res.results[c]["y"]
    return out


kernel.last_result = None
